# revision 17
# baseline (speedup 1.0000x reference)
"""GATv2 (nn_GATv2_59184649339075) Bass kernel for TRN2, 8-core SPMD.

Self-contained: kernel(**inputs) takes the full unsharded inputs
(x[50000,64], W[64,64], b[64], a[64], edge_index[2,800000] int32) and
returns the full [50000,64] float32 output.

Design (v4, group-uniform caps + 2x tree reductions):
  Host: nodes grouped into 400 dst-tiles of 128 by degree; each core owns
    50 tiles (tile rank r -> core r%8). Edges of a dst node occupy fixed
    columns: partition = dst pos, column = edge slot. Tiles are packed
    into column groups; within a group every tile shares uniform per-tile
    L/H caps (max over the group) so group-wide APs are regular.
    The f16 node table holds 256B rows [a~*Wh | Wh] (a~ = |a|, sign of a
    folded into a feature reorder: F+ first). int16 gather indices
    address two overlapping windows (L: rows<32768, H: rows>=18432);
    per-tile partition placement puts high-src-degree nodes in the
    overlap so per-edge window choice balances per-node L/H counts.
  Device per core, per rep, per group:
    gather 256B rows (4 dma_gather pieces) -> w[P, cols, 128]
    u-add: w[:,:,0:64] += whloc (2 group-wide bcast adds, 2x)
    prelu on ACT (sign-folded, in place)
    score tree: 64->32->16->8 adds (2x) + reduce8 (1x) -> e[P, cols]
    e += maskbias (-5 shift, -30000 for pad slots); exp on ACT into
    ex2[P, cols, 2] and into y66[:, :, 64] (denominator slot)
    y66[:,:,0:64] = w_raw * ex2-view (2x via 2-wide replication)
    aggregation: column-halving in-place trees (2x) over the L and H
    k-major blocks of y66 -> agg[P, 50, 66] (num 0:64, den 64)
    tail: rec = 1/max(den, eps); sigmoid(num * rec) on ACT.
"""
import sys

sys.path.insert(0, "/opt/trn_rl_repo")
from contextlib import ExitStack
from dataclasses import dataclass

import numpy as np

import concourse.bass as bass
import concourse.tile as tile
from concourse import bacc, mybir

F32 = mybir.dt.float32
F16 = mybir.dt.float16
I16 = mybir.dt.int16
AF = mybir.ActivationFunctionType
AX = mybir.AxisListType
OP = mybir.AluOpType

N_CORES = 8
P = 128
D = 64
NSLOPE = 0.2
N_TILES = 400
T_CORE = 50
NP_ = N_TILES * P          # 51200 padded nodes
LWIN = 32768               # L window rows [0, 32768)
HBASE = NP_ - 32768        # H window rows [18432, 51200)
GC = 112                   # max columns per edge group
SINGLE_PACKET = False      # SWDGE gather packet mode
GSPLIT = 2                 # gather pieces per window
ESHIFT = -5.0              # exp(e + ESHIFT): overflow headroom


@dataclass(frozen=True)
class Cfg:
    # per group: (k0, k1, cL, cH) with uniform caps cL/cH for k in [k0,k1)
    groups: tuple
    kpos: int                 # features with a >= 0 (F+ block size)


def wrap16(idx):
    n = len(idx)
    assert n % 16 == 0
    a = idx.reshape(n // 16, 16).T.astype(np.int16)
    return np.tile(a, (8, 1))


def prepare(x, W, b, a, edge_index):
    N = x.shape[0]
    E = edge_index.shape[1]
    src = edge_index[0].astype(np.int64)
    dst = edge_index[1].astype(np.int64)

    deg = np.bincount(dst, minlength=NP_)
    sdeg = np.bincount(src, minlength=NP_)

    # --- phase 1: table partition per node (src side) -------------------
    order0 = np.argsort(-deg, kind="stable")
    tiles0 = order0.reshape(N_TILES, P)
    node_part = np.empty(NP_, np.int64)
    OVER = np.arange(47, 81)
    OTHER = np.array([p for p in range(P) if not (47 <= p <= 80)])
    so = np.argsort(-sdeg[tiles0], axis=1, kind="stable")
    for t in range(N_TILES):
        m = tiles0[t]
        o = so[t]
        node_part[m[o[:34]]] = OVER
        node_part[m[o[34:]]] = OTHER

    # --- per-edge window assignment (balance L/H per dst node) ----------
    src_p = node_part[src]
    canL = src_p <= 80
    canH = src_p >= 47
    free = canL & canH
    nLh = np.bincount(dst[canL & ~free], minlength=NP_)
    nHh = np.bincount(dst[canH & ~free], minlength=NP_)
    nF = np.bincount(dst[free], minlength=NP_)
    tot = nLh + nHh + nF
    nLb = np.maximum(nLh, np.minimum(nLh + nF, (tot + 1) // 2))
    nHb = tot - nLb
    # free edges of each dst: first (nLb - nLh) go L, rest H
    eorder = np.lexsort((~free, dst))   # per dst: free edges first
    e_sorted = np.arange(E)[eorder]
    d_sorted = dst[eorder]
    first = np.r_[True, d_sorted[1:] != d_sorted[:-1]]
    starts = np.flatnonzero(first)
    rank = np.arange(E) - np.repeat(starts, np.diff(np.r_[starts, E]))
    isfree_s = free[e_sorted]
    quotaL = (nLb - nLh)[d_sorted]
    toL_s = np.where(isfree_s, rank < quotaL, canL[e_sorted])
    toL = np.empty(E, bool)
    toL[e_sorted] = toL_s
    assert (toL & ~canL).sum() == 0 and ((~toL) & ~canH).sum() == 0

    # --- phase 2: dst tiling + (pos, core) assignment -------------------
    key = np.lexsort((nHb, nLb, -tot))
    tiles = key.reshape(N_TILES, P)             # [tile, dstpos] -> node
    capL_t = nLb[tiles].max(1)
    capH_t = nHb[tiles].max(1)
    trank = np.argsort(-(capL_t + capH_t), kind="stable")
    # sorted position i -> pos i//8, core i%8
    tile_of = trank.reshape(T_CORE, N_CORES)     # [pos, core] -> tile id
    capL = capL_t[tile_of].max(1)
    capH = capH_t[tile_of].max(1)

    node_tile = np.empty(NP_, np.int64)
    node_dpos = np.empty(NP_, np.int64)
    for t in range(N_TILES):
        node_tile[tiles[t]] = t
        node_dpos[tiles[t]] = np.arange(P)

    # --- groups with uniform caps: DP to minimize padded columns --------
    # cost(i,j) = (j-i)*(maxL+maxH over [i,j)) + LAM, subject to <= GC
    LAM = 12
    INF = 1 << 40
    fdp = [INF] * (T_CORE + 1)
    fdp[T_CORE] = 0
    choice = [0] * T_CORE
    for i in range(T_CORE - 1, -1, -1):
        mL = 0
        mH = 0
        for j in range(i + 1, T_CORE + 1):
            mL = max(mL, int(capL[j - 1]))
            mH = max(mH, int(capH[j - 1]))
            w = (j - i) * (mL + mH)
            if w > GC and j > i + 1:
                break
            c = w + LAM + fdp[j]
            if c < fdp[i]:
                fdp[i] = c
                choice[i] = j
    groups = []
    k0 = 0
    while k0 < T_CORE:
        k1 = choice[k0]
        mL = int(max(capL[k0:k1]))
        mH = int(max(capH[k0:k1]))
        groups.append((k0, k1, mL, mH))
        k0 = k1
    cfg_groups = tuple(groups)
    ucapL = np.zeros(T_CORE, np.int64)
    ucapH = np.zeros(T_CORE, np.int64)
    for (k0g, k1g, cLg, cHg) in cfg_groups:
        ucapL[k0g:k1g] = cLg
        ucapH[k0g:k1g] = cHg

    # --- feature reorder + sign fold ------------------------------------
    pos_f = np.flatnonzero(a >= 0)
    neg_f = np.flatnonzero(a < 0)
    fperm = np.concatenate([pos_f, neg_f])
    kpos = len(pos_f)
    atil = np.abs(a)[fperm]
    Wp = W[fperm]                  # [64 out-perm, 64 in]
    bp = b[fperm]
    WT_aug = np.zeros((D + 1, 2 * D), np.float16)
    WT_aug[:D, 0:D] = (Wp.T * atil).astype(np.float16)
    WT_aug[:D, D:2 * D] = Wp.T.astype(np.float16)
    WT_aug[D, 0:D] = (bp * atil).astype(np.float16)
    WT_aug[D, D:2 * D] = bp.astype(np.float16)

    cfg = Cfg(groups=cfg_groups, kpos=kpos)

    # --- per-core data ---------------------------------------------------
    xpad = np.zeros((NP_, D), np.float32)
    xpad[:N] = x
    x16 = xpad.astype(np.float16)

    CC = int(sum((k1 - k0) * (cL + cH) for (k0, k1, cL, cH) in cfg_groups))

    in_maps = []
    # node table index t: per partition, nodes with that partition get
    # t = 0..399. Node's table column in xT = t*128+p.
    t_of = np.empty(NP_, np.int64)
    for p in range(P):
        nodes_p = np.flatnonzero(node_part == p)
        assert len(nodes_p) == N_TILES
        t_of[nodes_p] = np.arange(N_TILES)
    row = node_part * N_TILES + t_of            # table row
    xcol = np.empty(NP_, np.int64)
    xcol[t_of * P + node_part] = np.arange(NP_)
    xT = np.ascontiguousarray(x16[xcol].T)      # [64, 51200] f16
    xT_aug = np.concatenate([xT, np.ones((1, NP_), np.float16)])

    e_tile = node_tile[dst]
    tpos = np.empty(N_TILES, np.int64)
    tcore = np.empty(N_TILES, np.int64)
    for i in range(N_TILES):
        tcore[trank[i]] = i % N_CORES
        tpos[trank[i]] = i // N_CORES
    e_core = tcore[e_tile]
    e_pos = tpos[e_tile]
    e_dpos = node_dpos[dst]

    # column base offsets per (pos, window) in the group-local layout:
    # group layout = [L-block k-major | H-block k-major]
    colbaseL = np.zeros(T_CORE, np.int64)
    colbaseH = np.zeros(T_CORE, np.int64)
    off = 0
    for (k0g, k1g, cLg, cHg) in cfg_groups:
        nk = k1g - k0g
        for j, k in enumerate(range(k0g, k1g)):
            colbaseL[k] = off + j * cLg
        for j, k in enumerate(range(k0g, k1g)):
            colbaseH[k] = off + nk * cLg + j * cHg
        off += nk * (cLg + cHg)
    assert off == CC

    gstart = []
    off = 0
    for (k0g, k1g, cLg, cHg) in cfg_groups:
        nk = k1g - k0g
        gstart.append(off)
        off += nk * (cLg + cHg)

    # per-core rank of edge within (dst node, window)
    for c in range(N_CORES):
        m = e_core == c
        ed = dst[m]
        es = src[m]
        eL = toL[m]
        ep = e_pos[m]
        edp = e_dpos[m]
        okey = np.lexsort((es, ~eL, ed))
        dk = ed[okey]
        wk = eL[okey]
        bnd = np.r_[True, (dk[1:] != dk[:-1]) | (wk[1:] != wk[:-1])]
        st = np.flatnonzero(bnd)
        rk = np.arange(len(dk)) - np.repeat(st, np.diff(np.r_[st, len(dk)]))
        rank_e = np.empty(m.sum(), np.int64)
        rank_e[okey] = rk

        col = np.where(eL, colbaseL[ep] + rank_e, colbaseH[ep] + rank_e)
        slot = col * P + edp
        idx_full = np.zeros(CC * P, np.int64)          # default 0 (pad)
        r_e = row[es]
        idx_full[slot] = np.where(eL, r_e, r_e - HBASE)
        maskb = np.full(CC * P, -30000.0, np.float16)  # pad: exp -> 0
        maskb[slot] = ESHIFT

        idxL_parts = []
        idxH_parts = []
        for gi, (k0g, k1g, cLg, cHg) in enumerate(cfg_groups):
            nk = k1g - k0g
            s0 = gstart[gi] * P
            idxL_parts.append(wrap16(idx_full[s0:s0 + nk * cLg * P]))
            s1 = (gstart[gi] + nk * cLg) * P
            idxH_parts.append(wrap16(idx_full[s1:s1 + nk * cHg * P]))
        idxL = np.concatenate(idxL_parts, axis=1) if idxL_parts else \
            np.zeros((P, 0), np.int16)
        idxH = np.concatenate(idxH_parts, axis=1) if idxH_parts else \
            np.zeros((P, 0), np.int16)

        # local dst-tile x (dst-arranged): columns = (pos k, dpos p)
        own_nodes = tiles[tile_of[:, c]].reshape(-1)   # [50*128]
        xloc = np.ascontiguousarray(x16[own_nodes].T)
        xloc_aug = np.concatenate([xloc, np.ones((1, T_CORE * P), np.float16)])

        in_maps.append({
            "xT": xT_aug, "xTloc": xloc_aug, "WT": WT_aug,
            "idxL": idxL, "idxH": idxH,
            "maskb": np.ascontiguousarray(maskb.reshape(CC, P).T),
        })

    meta = {"N": N, "fperm": fperm, "tiles": tiles, "tile_of": tile_of,
            "cfg": cfg}
    return cfg, in_maps, meta


def build(cfg: Cfg, reps=1, stage="full"):
    # stage: ablation level — "gather", "score", "exp", "mult", "full"
    slvl = {"gather": 0, "score": 1, "exp": 2, "mult": 3, "full": 4}[stage]
    nc = bacc.Bacc("TRN2", target_bir_lowering=False, debug=False,
                   num_devices=N_CORES, num_swdge_queues=4)
    groups = cfg.groups
    kpos = cfg.kpos
    sumL = sum((k1 - k0) * cL for (k0, k1, cL, cH) in groups)
    sumH = sum((k1 - k0) * cH for (k0, k1, cL, cH) in groups)
    CC = sumL + sumH

    xT_d = nc.dram_tensor("xT", [D + 1, NP_], F16, kind="ExternalInput").ap()
    xTl_d = nc.dram_tensor("xTloc", [D + 1, T_CORE * P], F16,
                           kind="ExternalInput").ap()
    WT_d = nc.dram_tensor("WT", [D + 1, 2 * D], F16, kind="ExternalInput").ap()
    idxL_d = nc.dram_tensor("idxL", [P, sumL * 8], I16,
                            kind="ExternalInput").ap()
    idxH_d = nc.dram_tensor("idxH", [P, sumH * 8], I16,
                            kind="ExternalInput").ap()
    maskb_d = nc.dram_tensor("maskb", [P, CC], F16,
                             kind="ExternalInput").ap()
    out_d = nc.dram_tensor("out", [P, T_CORE * D], F16,
                           kind="ExternalOutput").ap()
    wh_t = nc.dram_tensor("wh", [P, N_TILES, 2 * D], F16)
    wh_d = wh_t.ap()
    wh_flat = wh_t.ap().rearrange("p t f -> (p t) f")

    with tile.TileContext(nc) as tc:
        with ExitStack() as ctx:
            cpool = ctx.enter_context(tc.tile_pool(name="const", bufs=1))
            WT_sb = cpool.tile([D + 1, 2 * D], F16)
            nc.sync.dma_start(WT_sb[:], WT_d[:, :])
            whloc = cpool.tile([P, T_CORE, D], F16)
            rpool = ctx.enter_context(tc.tile_pool(name="repstate", bufs=2))
            iL_sb = cpool.tile([P, sumL * 8], I16)
            nc.sync.dma_start(iL_sb[:], idxL_d[:, :])
            iH_sb = cpool.tile([P, sumH * 8], I16)
            nc.sync.dma_start(iH_sb[:], idxH_d[:, :])
            maskb_sb = cpool.tile([P, CC], F16)
            nc.sync.dma_start(maskb_sb[:], maskb_d[:, :])

            # ---- wh_stage: full table + local scaled tiles -------------
            with ExitStack() as c2:
                xp = c2.enter_context(tc.tile_pool(name="xt", bufs=3))
                pp = c2.enter_context(tc.tile_pool(name="whps", bufs=3,
                                                   space="PSUM"))
                sp = c2.enter_context(tc.tile_pool(name="whsb", bufs=3))
                GT = 8
                for g in range(N_TILES // GT):
                    t0 = g * GT
                    xt = xp.tile([D + 1, GT * P], F16, tag="xt")
                    nc.sync.dma_start(xt[:], xT_d[:, t0 * P:(t0 + GT) * P])
                    ps = pp.tile([P, GT, 2 * D], F32, tag="ps")
                    for j in range(GT):
                        nc.tensor.matmul(ps[:, j, :],
                                         lhsT=xt[:, j * P:(j + 1) * P],
                                         rhs=WT_sb[:], start=True, stop=True)
                    st = sp.tile([P, GT, 2 * D], F16, tag="st")
                    if g % 2 == 0:
                        nc.vector.tensor_copy(st[:], ps[:])
                    else:
                        nc.scalar.activation(st[:], ps[:], AF.Identity)
                    nc.sync.dma_start(wh_d[:, t0:t0 + GT, :], st[:])
                # local pass: 50 tiles dst-arranged, keep scaled half
                for g in range(7):
                    t0 = g * GT
                    nt = min(GT, T_CORE - t0)
                    xt = xp.tile([D + 1, GT * P], F16, tag="xt")
                    nc.sync.dma_start(xt[:, 0:nt * P],
                                      xTl_d[:, t0 * P:(t0 + nt) * P])
                    ps = pp.tile([P, GT, 2 * D], F32, tag="ps")
                    for j in range(nt):
                        nc.tensor.matmul(ps[:, j, :],
                                         lhsT=xt[:, j * P:(j + 1) * P],
                                         rhs=WT_sb[:], start=True, stop=True)
                    if g % 2 == 0:
                        nc.vector.tensor_copy(whloc[:, t0:t0 + nt, :],
                                              ps[:, 0:nt, 0:D])
                    else:
                        nc.scalar.activation(whloc[:, t0:t0 + nt, :],
                                             ps[:, 0:nt, 0:D], AF.Identity)

            gp = ctx.enter_context(tc.tile_pool(name="gath", bufs=4))
            tp1 = ctx.enter_context(tc.tile_pool(name="t1", bufs=1))
            tp2 = ctx.enter_context(tc.tile_pool(name="t2", bufs=1))
            tp3 = ctx.enter_context(tc.tile_pool(name="t3", bufs=1))
            ep_ = ctx.enter_context(tc.tile_pool(name="escore", bufs=2))
            xp2 = ctx.enter_context(tc.tile_pool(name="ex2", bufs=2))
            yp = ctx.enter_context(tc.tile_pool(name="y66", bufs=2))

            # per-group offsets
            goff = []
            offL, offH, offC = 0, 0, 0
            for (k0, k1, cL, cH) in groups:
                nk = k1 - k0
                goff.append((offL, offH, offC))
                offL += nk * cL
                offH += nk * cH
                offC += nk * (cL + cH)

            def emit_gather(gi):
                """issue the 4 dma_gather pieces for group gi."""
                k0, k1, cL, cH = groups[gi]
                if cL + cH == 0:
                    return None
                nk = k1 - k0
                oL, oH, oC = goff[gi]
                nL = nk * cL
                nH = nk * cH
                iL = iL_sb[:, oL * 8:(oL + nL) * 8]
                iH = iH_sb[:, oH * 8:(oH + nH) * 8]

                w = gp.tile([P, GC, 2 * D], F16, tag="w")
                q = gi % 4
                pieces = []
                for (base, cn, isL) in ((0, nL, True), (nL, nH, False)):
                    if cn == 0:
                        continue
                    ns = min(GSPLIT, cn)
                    bnd = [cn * i // ns for i in range(ns + 1)]
                    for i in range(ns):
                        if bnd[i + 1] > bnd[i]:
                            pieces.append((base + bnd[i], base + bnd[i + 1],
                                           bnd[i], isL))
                for (c0, c1, i0, isL) in pieces:
                    nn = c1 - c0
                    it = iL if isL else iH
                    src = wh_flat[0:LWIN, :] if isL else wh_flat[HBASE:NP_, :]
                    nc.gpsimd.dma_gather(
                        out_ap=w[:, c0:c1, :], in_ap=src,
                        idxs_ap=it[:, i0 * 8:(i0 + nn) * 8], num_idxs=nn * P,
                        num_idxs_reg=nn * P, elem_size=2 * D,
                        single_packet=SINGLE_PACKET, queue_num=q)
                    q = (q + 1) % 4
                return w

            def emit_part1(gi, st, w):
                """u-add + prelu for group gi (DVE then ACT)."""
                if st < 1 or w is None:
                    return
                k0, k1, cL, cH = groups[gi]
                nk = k1 - k0
                nL = nk * cL
                nH = nk * cH
                cols = nL + nH
                # u-add: w[:,:,0:64] += whloc broadcast (L block, H block)
                if nL:
                    wv = w[:, 0:nL, 0:D].rearrange("p (k c) f -> p k c f",
                                                   c=cL)
                    nc.vector.tensor_add(
                        wv, wv,
                        whloc[:, k0:k1, :].unsqueeze(2)
                        .to_broadcast((P, nk, cL, D)))
                if nH:
                    wv = w[:, nL:cols, 0:D].rearrange("p (k c) f -> p k c f",
                                                      c=cH)
                    nc.vector.tensor_add(
                        wv, wv,
                        whloc[:, k0:k1, :].unsqueeze(2)
                        .to_broadcast((P, nk, cH, D)))
                # sign-folded prelu:
                #   F+ (a>=0): Prelu_0.2(u);  F-: -Prelu_0.2(u)
                nc.scalar.activation(w[:, 0:cols, 0:kpos],
                                     w[:, 0:cols, 0:kpos],
                                     AF.Prelu, alpha=NSLOPE)
                nc.scalar.activation(w[:, 0:cols, kpos:D],
                                     w[:, 0:cols, kpos:D],
                                     AF.Prelu, alpha=1.0 / NSLOPE,
                                     scale=-NSLOPE)

            def emit_part2(gi, st, w):
                """score tree + mask + exp for group gi."""
                if st < 1 or w is None:
                    return (w, None, None)
                k0, k1, cL, cH = groups[gi]
                nk = k1 - k0
                oL, oH, oC = goff[gi]
                cols = nk * (cL + cH)
                # score tree: 64 -> 32 -> 16 -> 8 -> reduce
                t1 = tp1.tile([P, GC, 32], F16, tag="t1")
                nc.vector.tensor_add(t1[:, 0:cols, :], w[:, 0:cols, 0:32],
                                     w[:, 0:cols, 32:64])
                t2 = tp2.tile([P, GC, 16], F16, tag="t2")
                nc.vector.tensor_add(t2[:, 0:cols, :], t1[:, 0:cols, 0:16],
                                     t1[:, 0:cols, 16:32])
                t3 = tp3.tile([P, GC, 8], F16, tag="t3")
                nc.vector.tensor_add(t3[:, 0:cols, :], t2[:, 0:cols, 0:8],
                                     t2[:, 0:cols, 8:16])
                e = ep_.tile([P, GC], F16, tag="e")
                with nc.allow_low_precision(reason="f16 score sum"):
                    nc.vector.tensor_reduce(e[:, 0:cols], t3[:, 0:cols, :],
                                            axis=AX.X, op=OP.add)
                nc.vector.tensor_add(e[:, 0:cols], e[:, 0:cols],
                                     maskb_sb[:, oC:oC + cols])
                if st < 2:
                    return (w, None, None)
                ex2 = xp2.tile([P, GC, 2], F16, tag="ex2")
                nc.scalar.activation(
                    ex2[:, 0:cols, :],
                    e[:, 0:cols].unsqueeze(2).to_broadcast((P, cols, 2)),
                    AF.Exp)
                y = yp.tile([P, GC, D + 2], F16, tag="y")
                nc.scalar.activation(y[:, 0:cols, D], e[:, 0:cols], AF.Exp)
                return (w, ex2, y)

            cur_agg = [None]

            def emit_part3(gi, st, handles):
                """weighting mult + tree aggregation for group gi (DVE)."""
                k0, k1, cL, cH = groups[gi]
                if cL + cH == 0:
                    if st >= 4:
                        nc.vector.memset(cur_agg[0][:, k0:k1, :], 0.0)
                    return
                if st < 3 or handles is None or handles[1] is None:
                    return
                nk = k1 - k0
                nL = nk * cL
                nH = nk * cH
                cols = nL + nH
                w, ex2, y = handles
                agg = cur_agg[0]
                # y[:,:,0:64] = w_raw * ex (2x via 2-wide replicated view)
                exv = ex2[:, 0:cols, 0:2].unsqueeze(2) \
                    .to_broadcast((P, cols, 32, 2))
                nc.vector.tensor_mul(
                    y[:, 0:cols, 0:D].rearrange("p c (a b) -> p c a b", b=2),
                    w[:, 0:cols, D:2 * D].rearrange("p c (a b) -> p c a b",
                                                    b=2),
                    exv)
                if st < 4:
                    return
                # aggregation: in-place column-halving trees per block
                for (base, cap) in ((0, cL), (nL, cH)):
                    if cap == 0:
                        continue
                    blk = y[:, base:base + nk * cap, :].rearrange(
                        "p (k c) f -> p k c f", c=cap)
                    h = cap
                    while h > 1:
                        lo = (h + 1) // 2
                        nc.vector.tensor_add(blk[:, :, 0:h - lo, :],
                                             blk[:, :, 0:h - lo, :],
                                             blk[:, :, lo:h, :])
                        h = lo
                # combine L + H roots -> agg[:, k0:k1, :]
                yL = y[:, 0:nL, :].rearrange("p (k c) f -> p k c f", c=cL) \
                    if nL else None
                yH = y[:, nL:cols, :].rearrange("p (k c) f -> p k c f", c=cH) \
                    if nH else None
                if yL is not None and yH is not None:
                    nc.vector.tensor_add(agg[:, k0:k1, :], yL[:, :, 0, :],
                                         yH[:, :, 0, :])
                elif yL is not None:
                    nc.vector.tensor_copy(agg[:, k0:k1, :], yL[:, :, 0, :])
                else:
                    nc.vector.tensor_copy(agg[:, k0:k1, :], yH[:, :, 0, :])

            ng = len(groups)
            for rep in range(reps):
                agg = rpool.tile([P, T_CORE, D + 2], F16, tag="agg")
                cur_agg[0] = agg
                gw = {0: emit_gather(0)}
                if ng > 1:
                    gw[1] = emit_gather(1)
                handles = {}
                for s in range(ng):
                    if s + 2 < ng:
                        gw[s + 2] = emit_gather(s + 2)
                    emit_part1(s, slvl, gw[s])
                    if s >= 1:
                        emit_part3(s - 1, slvl, handles.pop(s - 1))
                    handles[s] = emit_part2(s, slvl, gw.pop(s))
                emit_part3(ng - 1, slvl, handles.pop(ng - 1))
                if slvl < 4:
                    continue

                # tail: rec = 1 / max(den, eps); out = sigmoid(num * rec)
                rec = rpool.tile([P, T_CORE], F32, tag="rec")
                rec2 = rpool.tile([P, T_CORE, 2], F16, tag="rec2")
                obuf = rpool.tile([P, T_CORE, D], F16, tag="obuf")
                nc.vector.tensor_scalar_max(rec[:], agg[:, :, D], 1e-9)
                nc.vector.reciprocal(rec[:], rec[:])
                nc.scalar.activation(
                    rec2[:],
                    rec[:].unsqueeze(2).to_broadcast((P, T_CORE, 2)),
                    AF.Identity)
                nc.vector.tensor_mul(
                    obuf[:].rearrange("p t (a b) -> p t a b", b=2),
                    agg[:, :, 0:D].rearrange("p t (a b) -> p t a b", b=2),
                    rec2[:, :, 0:2].unsqueeze(2)
                    .to_broadcast((P, T_CORE, 32, 2)))
                nc.scalar.activation(obuf[:], obuf[:], AF.Sigmoid)
                nc.sync.dma_start(
                    out_d[:, :], obuf[:].rearrange("p t f -> p (t f)"))

    nc.compile()
    return nc


_CACHE = {}


def kernel(x, W, b, a, edge_index):
    x = np.ascontiguousarray(np.asarray(x, dtype=np.float32))
    W = np.ascontiguousarray(np.asarray(W, dtype=np.float32))
    b = np.ascontiguousarray(np.asarray(b, dtype=np.float32))
    a = np.ascontiguousarray(np.asarray(a, dtype=np.float32))
    edge_index = np.asarray(edge_index)

    cfg, in_maps, meta = prepare(x, W, b, a, edge_index)
    nc = _CACHE.get(cfg)
    if nc is None:
        nc = build(cfg)
        _CACHE[cfg] = nc

    from concourse.bass_utils import run_bass_kernel_spmd
    res = run_bass_kernel_spmd(nc, in_maps, core_ids=list(range(N_CORES)))

    N = meta["N"]
    fperm = meta["fperm"]
    tiles = meta["tiles"]
    tile_of = meta["tile_of"]
    inv_f = np.argsort(fperm)
    y = np.empty((NP_, D), np.float32)
    for c in range(N_CORES):
        o = np.asarray(res.results[c]["out"]).reshape(P, T_CORE, D)
        own = tiles[tile_of[:, c]]              # [50, 128]; o[p,k]=own[k,p]
        y[own.transpose(1, 0).reshape(-1)] = o.reshape(-1, D)
    return y[:N][:, inv_f].astype(np.float32)


# revision 20
# speedup vs baseline: 1.9489x; 1.9489x over previous
"""GATv2 (nn_GATv2_59184649339075) Bass kernel for TRN2, 8-core SPMD.

Self-contained: kernel(**inputs) takes the full unsharded inputs
(x[50000,64], W[64,64], b[64], a[64], edge_index[2,800000] int32) and
returns the full [50000,64] float32 output.

Design (v5, 128B scaled-only gather + group-uniform caps + 2x trees):
  Host: nodes grouped into 400 dst-tiles of 128 by degree; each core owns
    50 tiles (tile rank r -> core r%8). Edges of a dst node occupy fixed
    columns: partition = dst pos, column = edge slot. Tiles are packed
    into column groups (DP minimizing padded columns); within a group
    every tile shares uniform per-tile L/H caps so group-wide APs are
    regular. The f16 node table holds 256B rows [a~*Wh | Wh] (a~ = |a|,
    sign of a folded into a feature reorder: F+ first); the edge phase
    gathers only the scaled 128B half (elem_size=64, elem_step=128 via
    dma_gather_sub) — the unscaled numerator is recovered at the output
    by multiplying the aggregated tensor by 1/a~ (exact: num = a~*num').
    int16 gather indices address two overlapping windows (L: rows<32768,
    H: rows>=18432); per-tile partition placement puts high-src-degree
    nodes in the overlap so per-edge window choice balances per-node L/H
    counts. Pad slots gather random rows (a fixed pad row would pile all
    pad descriptors onto one HBM bank and serialize them).
  Device per core, per rep, per group:
    gather 128B scaled rows (4 dma_gather pieces) -> w[P, cols, 64]
    u = w + whloc (2 group-wide bcast adds, 2x); prelu on ACT in place
    score tree: 64->32->...->2->1 adds (2x) -> e[P, cols]
    e += maskbias (-5 shift, -30000 for pad slots); exp on ACT into
    ex2[P, cols, 8] and into y66[:, :, 64] (denominator slot)
    y66[:,:,0:64] = w * ex (2x via 8-wide replicated ex view)
    aggregation: column-halving in-place trees (2x) over the L and H
    k-major blocks of y66 -> agg[P, 50, 66] (num' 0:64, den 64)
    tail: rec = 1/max(den, eps); obuf = num' * rec * (1/a~) (2x);
    one sigmoid over [P, 50, 64] on ACT.
"""
import sys

sys.path.insert(0, "/opt/trn_rl_repo")
from contextlib import ExitStack
from dataclasses import dataclass

import numpy as np

import concourse.bass as bass
import concourse.tile as tile
from concourse import bacc, mybir

F32 = mybir.dt.float32
F16 = mybir.dt.float16
I16 = mybir.dt.int16
AF = mybir.ActivationFunctionType
AX = mybir.AxisListType
OP = mybir.AluOpType

N_CORES = 8
P = 128
D = 64
NSLOPE = 0.2
N_TILES = 400
T_CORE = 50
NP_ = N_TILES * P          # 51200 padded nodes
LWIN = 32768               # L window rows [0, 32768)
HBASE = NP_ - 32768        # H window rows [18432, 51200)
GC = 112                   # max columns per edge group
SINGLE_PACKET = False      # SWDGE gather packet mode
GSPLIT = 2                 # gather pieces per window
ESHIFT = -5.0              # exp(e + ESHIFT): overflow headroom
EXW = 8                    # exp replication width for the weighting mult


def dma_gather_sub(eng, out_ap, in_ap, idxs_ap, num_idxs, elem_size,
                   elem_step, single_packet, queue_num):
    """dma_gather with sub-row payload (elem_size_bytes % 256 != 0 but
    row stride % 256 == 0). Mirrors BassGpSimd.dma_gather minus the
    over-broad 256B elem_size assert (a transpose-path restriction)."""
    from concourse import ap_utils
    from concourse.bass import MemorySpace
    from concourse._compat import exact_div

    assert idxs_ap.dtype == mybir.dt.int16
    assert in_ap.space == MemorySpace.DRAM
    assert idxs_ap.space == MemorySpace.SBUF
    assert out_ap.space == MemorySpace.SBUF
    assert ap_utils.ap_is_contiguous(in_ap.ap[1:])
    assert ap_utils.ap_is_contiguous(out_ap.ap[1:])
    assert ap_utils.ap_is_contiguous(idxs_ap.ap[1:])
    assert in_ap.ap[-1][1] == out_ap.ap[-1][1] == elem_size
    assert out_ap.ap[0][1] * out_ap.ap[1][1] % 128 == 0
    assert in_ap.ap[0][0] == elem_step
    stride_bytes = elem_step * mybir.dt.size(in_ap.dtype)
    stride_bytes_256 = exact_div(stride_bytes, 256)
    assert stride_bytes_256 < 256
    _in_ap = eng.lower_ap_dma(in_ap, for_custom_bir_dma=True)
    inst = eng.add_instruction(
        mybir.InstDMAGatherAnt(
            name=eng.bass.get_next_instruction_name(),
            ins=[
                *_in_ap,
                eng.lower_ap(idxs_ap),
                eng.lower_val_access(eng.to_reg(num_idxs)),
            ],
            outs=[eng.lower_ap(out_ap)],
            transpose=False,
            num_idxs=num_idxs,
            elem_size=elem_size,
            stride_bytes_256=stride_bytes_256,
            gen_mode=0,
            single_packet=single_packet,
            queue_num=queue_num,
            sbuf_tokens_per_rank=0,
            sbuf_free_dim_per_rank=0,
            sbuf_free_dim_pad_per_rank=0,
            sbuf_byte_offset=0,
        )
    )
    return inst


@dataclass(frozen=True)
class Cfg:
    # per group: (k0, k1, cL, cH) with uniform caps cL/cH for k in [k0,k1)
    groups: tuple
    kpos: int                 # features with a >= 0 (F+ block size)


def wrap16(idx):
    n = len(idx)
    assert n % 16 == 0
    a = idx.reshape(n // 16, 16).T.astype(np.int16)
    return np.tile(a, (8, 1))


def prepare(x, W, b, a, edge_index):
    N = x.shape[0]
    E = edge_index.shape[1]
    src = edge_index[0].astype(np.int64)
    dst = edge_index[1].astype(np.int64)

    deg = np.bincount(dst, minlength=NP_)
    sdeg = np.bincount(src, minlength=NP_)

    # --- phase 1: table partition per node (src side) -------------------
    order0 = np.argsort(-deg, kind="stable")
    tiles0 = order0.reshape(N_TILES, P)
    node_part = np.empty(NP_, np.int64)
    OVER = np.arange(47, 81)
    OTHER = np.array([p for p in range(P) if not (47 <= p <= 80)])
    so = np.argsort(-sdeg[tiles0], axis=1, kind="stable")
    for t in range(N_TILES):
        m = tiles0[t]
        o = so[t]
        node_part[m[o[:34]]] = OVER
        node_part[m[o[34:]]] = OTHER

    # --- per-edge window assignment (balance L/H per dst node) ----------
    src_p = node_part[src]
    canL = src_p <= 80
    canH = src_p >= 47
    free = canL & canH
    nLh = np.bincount(dst[canL & ~free], minlength=NP_)
    nHh = np.bincount(dst[canH & ~free], minlength=NP_)
    nF = np.bincount(dst[free], minlength=NP_)
    tot = nLh + nHh + nF
    nLb = np.maximum(nLh, np.minimum(nLh + nF, (tot + 1) // 2))
    nHb = tot - nLb
    # free edges of each dst: first (nLb - nLh) go L, rest H
    eorder = np.lexsort((~free, dst))   # per dst: free edges first
    e_sorted = np.arange(E)[eorder]
    d_sorted = dst[eorder]
    first = np.r_[True, d_sorted[1:] != d_sorted[:-1]]
    starts = np.flatnonzero(first)
    rank = np.arange(E) - np.repeat(starts, np.diff(np.r_[starts, E]))
    isfree_s = free[e_sorted]
    quotaL = (nLb - nLh)[d_sorted]
    toL_s = np.where(isfree_s, rank < quotaL, canL[e_sorted])
    toL = np.empty(E, bool)
    toL[e_sorted] = toL_s
    assert (toL & ~canL).sum() == 0 and ((~toL) & ~canH).sum() == 0

    # --- phase 2: dst tiling + (pos, core) assignment -------------------
    key = np.lexsort((nHb, nLb, -tot))
    tiles = key.reshape(N_TILES, P)             # [tile, dstpos] -> node
    capL_t = nLb[tiles].max(1)
    capH_t = nHb[tiles].max(1)
    trank = np.argsort(-(capL_t + capH_t), kind="stable")
    # sorted position i -> pos i//8, core i%8
    tile_of = trank.reshape(T_CORE, N_CORES)     # [pos, core] -> tile id
    capL = capL_t[tile_of].max(1)
    capH = capH_t[tile_of].max(1)

    node_tile = np.empty(NP_, np.int64)
    node_dpos = np.empty(NP_, np.int64)
    for t in range(N_TILES):
        node_tile[tiles[t]] = t
        node_dpos[tiles[t]] = np.arange(P)

    # --- groups with uniform caps: DP to minimize padded columns --------
    # cost(i,j) = (j-i)*(maxL+maxH over [i,j)) + LAM, subject to <= GC
    LAM = 24
    INF = 1 << 40
    fdp = [INF] * (T_CORE + 1)
    fdp[T_CORE] = 0
    choice = [0] * T_CORE
    for i in range(T_CORE - 1, -1, -1):
        mL = 0
        mH = 0
        for j in range(i + 1, T_CORE + 1):
            mL = max(mL, int(capL[j - 1]))
            mH = max(mH, int(capH[j - 1]))
            w = (j - i) * (mL + mH)
            if w > GC and j > i + 1:
                break
            c = w + LAM + fdp[j]
            if c < fdp[i]:
                fdp[i] = c
                choice[i] = j
    groups = []
    k0 = 0
    while k0 < T_CORE:
        k1 = choice[k0]
        mL = int(max(capL[k0:k1]))
        mH = int(max(capH[k0:k1]))
        groups.append((k0, k1, mL, mH))
        k0 = k1
    cfg_groups = tuple(groups)
    ucapL = np.zeros(T_CORE, np.int64)
    ucapH = np.zeros(T_CORE, np.int64)
    for (k0g, k1g, cLg, cHg) in cfg_groups:
        ucapL[k0g:k1g] = cLg
        ucapH[k0g:k1g] = cHg

    # --- feature reorder + sign fold ------------------------------------
    pos_f = np.flatnonzero(a >= 0)
    neg_f = np.flatnonzero(a < 0)
    fperm = np.concatenate([pos_f, neg_f])
    kpos = len(pos_f)
    atil = np.abs(a)[fperm]
    Wp = W[fperm]                  # [64 out-perm, 64 in]
    bp = b[fperm]
    WT_aug = np.zeros((D + 1, 2 * D), np.float16)
    WT_aug[:D, 0:D] = (Wp.T * atil).astype(np.float16)
    WT_aug[:D, D:2 * D] = Wp.T.astype(np.float16)
    WT_aug[D, 0:D] = (bp * atil).astype(np.float16)
    WT_aug[D, D:2 * D] = bp.astype(np.float16)

    cfg = Cfg(groups=cfg_groups, kpos=kpos)

    # --- per-core data ---------------------------------------------------
    xpad = np.zeros((NP_, D), np.float32)
    xpad[:N] = x
    x16 = xpad.astype(np.float16)

    CC = int(sum((k1 - k0) * (cL + cH) for (k0, k1, cL, cH) in cfg_groups))

    in_maps = []
    # node table index t: per partition, nodes with that partition get
    # t = 0..399. Node's table column in xT = t*128+p.
    t_of = np.empty(NP_, np.int64)
    for p in range(P):
        nodes_p = np.flatnonzero(node_part == p)
        assert len(nodes_p) == N_TILES
        t_of[nodes_p] = np.arange(N_TILES)
    row = node_part * N_TILES + t_of            # table row
    xcol = np.empty(NP_, np.int64)
    xcol[t_of * P + node_part] = np.arange(NP_)
    xT = np.ascontiguousarray(x16[xcol].T)      # [64, 51200] f16
    xT_aug = np.concatenate([xT, np.ones((1, NP_), np.float16)])

    e_tile = node_tile[dst]
    tpos = np.empty(N_TILES, np.int64)
    tcore = np.empty(N_TILES, np.int64)
    for i in range(N_TILES):
        tcore[trank[i]] = i % N_CORES
        tpos[trank[i]] = i // N_CORES
    e_core = tcore[e_tile]
    e_pos = tpos[e_tile]
    e_dpos = node_dpos[dst]

    # column base offsets per (pos, window) in the group-local layout:
    # group layout = [L-block k-major | H-block k-major]
    colbaseL = np.zeros(T_CORE, np.int64)
    colbaseH = np.zeros(T_CORE, np.int64)
    off = 0
    for (k0g, k1g, cLg, cHg) in cfg_groups:
        nk = k1g - k0g
        for j, k in enumerate(range(k0g, k1g)):
            colbaseL[k] = off + j * cLg
        for j, k in enumerate(range(k0g, k1g)):
            colbaseH[k] = off + nk * cLg + j * cHg
        off += nk * (cLg + cHg)
    assert off == CC

    gstart = []
    off = 0
    for (k0g, k1g, cLg, cHg) in cfg_groups:
        nk = k1g - k0g
        gstart.append(off)
        off += nk * (cLg + cHg)

    # per-core rank of edge within (dst node, window)
    for c in range(N_CORES):
        m = e_core == c
        ed = dst[m]
        es = src[m]
        eL = toL[m]
        ep = e_pos[m]
        edp = e_dpos[m]
        okey = np.lexsort((es, ~eL, ed))
        dk = ed[okey]
        wk = eL[okey]
        bnd = np.r_[True, (dk[1:] != dk[:-1]) | (wk[1:] != wk[:-1])]
        st = np.flatnonzero(bnd)
        rk = np.arange(len(dk)) - np.repeat(st, np.diff(np.r_[st, len(dk)]))
        rank_e = np.empty(m.sum(), np.int64)
        rank_e[okey] = rk

        col = np.where(eL, colbaseL[ep] + rank_e, colbaseH[ep] + rank_e)
        slot = col * P + edp
        # pad slots gather random (masked) rows: a constant pad row would
        # focus all pad descriptors on one HBM bank and serialize them
        rng = np.random.default_rng(0xC0DE + c)
        idx_full = rng.integers(0, 32768, CC * P).astype(np.int64)
        r_e = row[es]
        idx_full[slot] = np.where(eL, r_e, r_e - HBASE)
        maskb = np.full(CC * P, -30000.0, np.float16)  # pad: exp -> 0
        maskb[slot] = ESHIFT

        idxL_parts = []
        idxH_parts = []
        for gi, (k0g, k1g, cLg, cHg) in enumerate(cfg_groups):
            nk = k1g - k0g
            s0 = gstart[gi] * P
            idxL_parts.append(wrap16(idx_full[s0:s0 + nk * cLg * P]))
            s1 = (gstart[gi] + nk * cLg) * P
            idxH_parts.append(wrap16(idx_full[s1:s1 + nk * cHg * P]))
        idxL = np.concatenate(idxL_parts, axis=1) if idxL_parts else \
            np.zeros((P, 0), np.int16)
        idxH = np.concatenate(idxH_parts, axis=1) if idxH_parts else \
            np.zeros((P, 0), np.int16)

        # local dst-tile x (dst-arranged): columns = (pos k, dpos p)
        own_nodes = tiles[tile_of[:, c]].reshape(-1)   # [50*128]
        xloc = np.ascontiguousarray(x16[own_nodes].T)
        xloc_aug = np.concatenate([xloc, np.ones((1, T_CORE * P), np.float16)])

        inva = (1.0 / np.maximum(atil, 1e-8)).astype(np.float16)
        in_maps.append({
            "xT": xT_aug, "xTloc": xloc_aug, "WT": WT_aug,
            "idxL": idxL, "idxH": idxH,
            "maskb": np.ascontiguousarray(maskb.reshape(CC, P).T),
            "inva": np.ascontiguousarray(np.tile(inva, (P, 1))),
        })

    meta = {"N": N, "fperm": fperm, "tiles": tiles, "tile_of": tile_of,
            "cfg": cfg}
    return cfg, in_maps, meta


def build(cfg: Cfg, reps=1, stage="full"):
    # stage: ablation level — "gather", "score", "exp", "mult", "full"
    slvl = {"gather": 0, "score": 1, "exp": 2, "mult": 3, "full": 4}[stage]
    nc = bacc.Bacc("TRN2", target_bir_lowering=False, debug=False,
                   num_devices=N_CORES, num_swdge_queues=4)
    groups = cfg.groups
    kpos = cfg.kpos
    sumL = sum((k1 - k0) * cL for (k0, k1, cL, cH) in groups)
    sumH = sum((k1 - k0) * cH for (k0, k1, cL, cH) in groups)
    CC = sumL + sumH

    xT_d = nc.dram_tensor("xT", [D + 1, NP_], F16, kind="ExternalInput").ap()
    xTl_d = nc.dram_tensor("xTloc", [D + 1, T_CORE * P], F16,
                           kind="ExternalInput").ap()
    WT_d = nc.dram_tensor("WT", [D + 1, 2 * D], F16, kind="ExternalInput").ap()
    idxL_d = nc.dram_tensor("idxL", [P, sumL * 8], I16,
                            kind="ExternalInput").ap()
    idxH_d = nc.dram_tensor("idxH", [P, sumH * 8], I16,
                            kind="ExternalInput").ap()
    maskb_d = nc.dram_tensor("maskb", [P, CC], F16,
                             kind="ExternalInput").ap()
    inva_d = nc.dram_tensor("inva", [P, D], F16, kind="ExternalInput").ap()
    out_d = nc.dram_tensor("out", [P, T_CORE * D], F16,
                           kind="ExternalOutput").ap()
    wh_t = nc.dram_tensor("wh", [P, N_TILES, 2 * D], F16)
    wh_d = wh_t.ap()
    wh_flat = wh_t.ap().rearrange("p t f -> (p t) f")

    with tile.TileContext(nc) as tc:
        with ExitStack() as ctx:
            cpool = ctx.enter_context(tc.tile_pool(name="const", bufs=1))
            WT_sb = cpool.tile([D + 1, 2 * D], F16)
            nc.sync.dma_start(WT_sb[:], WT_d[:, :])
            whloc = cpool.tile([P, T_CORE, D], F16)
            rpool = ctx.enter_context(tc.tile_pool(name="repstate", bufs=2))
            iL_sb = cpool.tile([P, sumL * 8], I16)
            nc.sync.dma_start(iL_sb[:], idxL_d[:, :])
            iH_sb = cpool.tile([P, sumH * 8], I16)
            nc.sync.dma_start(iH_sb[:], idxH_d[:, :])
            maskb_sb = cpool.tile([P, CC], F16)
            nc.sync.dma_start(maskb_sb[:], maskb_d[:, :])
            inva_sb = cpool.tile([P, D], F16)
            nc.sync.dma_start(inva_sb[:], inva_d[:, :])

            # ---- wh_stage: full table + local scaled tiles -------------
            with ExitStack() as c2:
                xp = c2.enter_context(tc.tile_pool(name="xt", bufs=3))
                pp = c2.enter_context(tc.tile_pool(name="whps", bufs=3,
                                                   space="PSUM"))
                sp = c2.enter_context(tc.tile_pool(name="whsb", bufs=3))
                GT = 8
                for g in range(N_TILES // GT):
                    t0 = g * GT
                    xt = xp.tile([D + 1, GT * P], F16, tag="xt")
                    nc.sync.dma_start(xt[:], xT_d[:, t0 * P:(t0 + GT) * P])
                    ps = pp.tile([P, GT, 2 * D], F32, tag="ps")
                    for j in range(GT):
                        nc.tensor.matmul(ps[:, j, :],
                                         lhsT=xt[:, j * P:(j + 1) * P],
                                         rhs=WT_sb[:], start=True, stop=True)
                    st = sp.tile([P, GT, 2 * D], F16, tag="st")
                    if g % 2 == 0:
                        nc.vector.tensor_copy(st[:], ps[:])
                    else:
                        nc.scalar.activation(st[:], ps[:], AF.Identity)
                    nc.sync.dma_start(wh_d[:, t0:t0 + GT, :], st[:])
                # local pass: 50 tiles dst-arranged, keep scaled half
                for g in range(7):
                    t0 = g * GT
                    nt = min(GT, T_CORE - t0)
                    xt = xp.tile([D + 1, GT * P], F16, tag="xt")
                    nc.sync.dma_start(xt[:, 0:nt * P],
                                      xTl_d[:, t0 * P:(t0 + nt) * P])
                    ps = pp.tile([P, GT, 2 * D], F32, tag="ps")
                    for j in range(nt):
                        nc.tensor.matmul(ps[:, j, :],
                                         lhsT=xt[:, j * P:(j + 1) * P],
                                         rhs=WT_sb[:], start=True, stop=True)
                    if g % 2 == 0:
                        nc.vector.tensor_copy(whloc[:, t0:t0 + nt, :],
                                              ps[:, 0:nt, 0:D])
                    else:
                        nc.scalar.activation(whloc[:, t0:t0 + nt, :],
                                             ps[:, 0:nt, 0:D], AF.Identity)

            gp = ctx.enter_context(tc.tile_pool(name="gath", bufs=4))
            up = ctx.enter_context(tc.tile_pool(name="u", bufs=2))
            tp1 = ctx.enter_context(tc.tile_pool(name="t1", bufs=1))
            tp2 = ctx.enter_context(tc.tile_pool(name="t2", bufs=1))
            tp3 = ctx.enter_context(tc.tile_pool(name="t3", bufs=1))
            ep_ = ctx.enter_context(tc.tile_pool(name="escore", bufs=2))
            xp2 = ctx.enter_context(tc.tile_pool(name="ex2", bufs=2))
            yp = ctx.enter_context(tc.tile_pool(name="y66", bufs=2))

            # per-group offsets
            goff = []
            offL, offH, offC = 0, 0, 0
            for (k0, k1, cL, cH) in groups:
                nk = k1 - k0
                goff.append((offL, offH, offC))
                offL += nk * cL
                offH += nk * cH
                offC += nk * (cL + cH)

            def emit_gather(gi):
                """issue the 4 dma_gather pieces for group gi."""
                k0, k1, cL, cH = groups[gi]
                if cL + cH == 0:
                    return None
                nk = k1 - k0
                oL, oH, oC = goff[gi]
                nL = nk * cL
                nH = nk * cH
                iL = iL_sb[:, oL * 8:(oL + nL) * 8]
                iH = iH_sb[:, oH * 8:(oH + nH) * 8]

                w = gp.tile([P, GC, D], F16, tag="w")
                q = gi % 4
                pieces = []
                for (base, cn, isL) in ((0, nL, True), (nL, nH, False)):
                    if cn == 0:
                        continue
                    ns = min(GSPLIT, cn)
                    bnd = [cn * i // ns for i in range(ns + 1)]
                    for i in range(ns):
                        if bnd[i + 1] > bnd[i]:
                            pieces.append((base + bnd[i], base + bnd[i + 1],
                                           bnd[i], isL))
                for (c0, c1, i0, isL) in pieces:
                    nn = c1 - c0
                    it = iL if isL else iH
                    src = wh_flat[0:LWIN, 0:D] if isL \
                        else wh_flat[HBASE:NP_, 0:D]
                    dma_gather_sub(
                        nc.gpsimd, out_ap=w[:, c0:c1, :], in_ap=src,
                        idxs_ap=it[:, i0 * 8:(i0 + nn) * 8], num_idxs=nn * P,
                        elem_size=D, elem_step=2 * D,
                        single_packet=SINGLE_PACKET, queue_num=q)
                    q = (q + 1) % 4
                return w

            def emit_part1(gi, st, w):
                """u-add + prelu for group gi (DVE then ACT)."""
                if st < 1 or w is None:
                    return None
                k0, k1, cL, cH = groups[gi]
                nk = k1 - k0
                nL = nk * cL
                nH = nk * cH
                cols = nL + nH
                # u = w + whloc broadcast (L block, H block); w stays raw
                u = up.tile([P, GC, D], F16, tag="u")
                if nL:
                    uv = u[:, 0:nL, :].rearrange("p (k c) f -> p k c f",
                                                 c=cL)
                    wv = w[:, 0:nL, :].rearrange("p (k c) f -> p k c f",
                                                 c=cL)
                    nc.vector.tensor_add(
                        uv, wv,
                        whloc[:, k0:k1, :].unsqueeze(2)
                        .to_broadcast((P, nk, cL, D)))
                if nH:
                    uv = u[:, nL:cols, :].rearrange("p (k c) f -> p k c f",
                                                    c=cH)
                    wv = w[:, nL:cols, :].rearrange("p (k c) f -> p k c f",
                                                    c=cH)
                    nc.vector.tensor_add(
                        uv, wv,
                        whloc[:, k0:k1, :].unsqueeze(2)
                        .to_broadcast((P, nk, cH, D)))
                # sign-folded prelu:
                #   F+ (a>=0): Prelu_0.2(u);  F-: -Prelu_0.2(u)
                nc.scalar.activation(u[:, 0:cols, 0:kpos],
                                     u[:, 0:cols, 0:kpos],
                                     AF.Prelu, alpha=NSLOPE)
                nc.scalar.activation(u[:, 0:cols, kpos:D],
                                     u[:, 0:cols, kpos:D],
                                     AF.Prelu, alpha=1.0 / NSLOPE,
                                     scale=-NSLOPE)
                return u

            def emit_part2(gi, st, w, u):
                """score tree + mask + exp for group gi."""
                if st < 1 or w is None:
                    return (w, None, None)
                k0, k1, cL, cH = groups[gi]
                nk = k1 - k0
                oL, oH, oC = goff[gi]
                cols = nk * (cL + cH)
                # score tree: 64 -> 32 -> 16 -> 8 -> reduce
                t1 = tp1.tile([P, GC, 32], F16, tag="t1")
                nc.vector.tensor_add(t1[:, 0:cols, :], u[:, 0:cols, 0:32],
                                     u[:, 0:cols, 32:64])
                t2 = tp2.tile([P, GC, 16], F16, tag="t2")
                nc.vector.tensor_add(t2[:, 0:cols, :], t1[:, 0:cols, 0:16],
                                     t1[:, 0:cols, 16:32])
                t3 = tp3.tile([P, GC, 8], F16, tag="t3")
                nc.vector.tensor_add(t3[:, 0:cols, :], t2[:, 0:cols, 0:8],
                                     t2[:, 0:cols, 8:16])
                t4 = tp3.tile([P, GC, 4], F16, tag="t4")
                nc.vector.tensor_add(t4[:, 0:cols, :], t3[:, 0:cols, 0:4],
                                     t3[:, 0:cols, 4:8])
                t5 = tp3.tile([P, GC, 2], F16, tag="t5")
                nc.vector.tensor_add(t5[:, 0:cols, :], t4[:, 0:cols, 0:2],
                                     t4[:, 0:cols, 2:4])
                e = ep_.tile([P, GC], F16, tag="e")
                nc.vector.tensor_add(e[:, 0:cols], t5[:, 0:cols, 0],
                                     t5[:, 0:cols, 1])
                nc.vector.tensor_add(e[:, 0:cols], e[:, 0:cols],
                                     maskb_sb[:, oC:oC + cols])
                if st < 2:
                    return (w, None, None)
                ex2 = xp2.tile([P, GC, EXW], F16, tag="ex2")
                nc.scalar.activation(
                    ex2[:, 0:cols, :],
                    e[:, 0:cols].unsqueeze(2).to_broadcast((P, cols, EXW)),
                    AF.Exp)
                y = yp.tile([P, GC, D + 2], F16, tag="y")
                nc.scalar.activation(y[:, 0:cols, D], e[:, 0:cols], AF.Exp)
                return (w, ex2, y)

            cur_agg = [None]

            def emit_part3(gi, st, handles):
                """weighting mult + tree aggregation for group gi (DVE)."""
                k0, k1, cL, cH = groups[gi]
                if cL + cH == 0:
                    if st >= 4:
                        nc.vector.memset(cur_agg[0][:, k0:k1, :], 0.0)
                    return
                if st < 3 or handles is None or handles[1] is None:
                    return
                nk = k1 - k0
                nL = nk * cL
                nH = nk * cH
                cols = nL + nH
                w, ex2, y = handles
                agg = cur_agg[0]
                # y[:,:,0:64] = w_scaled * ex (2x via EXW-replicated view)
                exv = ex2[:, 0:cols, 0:EXW].unsqueeze(2) \
                    .to_broadcast((P, cols, D // EXW, EXW))
                nc.vector.tensor_mul(
                    y[:, 0:cols, 0:D].rearrange("p c (a b) -> p c a b",
                                                b=EXW),
                    w[:, 0:cols, :].rearrange("p c (a b) -> p c a b", b=EXW),
                    exv)
                if st < 4:
                    return
                # aggregation: in-place column-halving trees per block
                for (base, cap) in ((0, cL), (nL, cH)):
                    if cap == 0:
                        continue
                    blk = y[:, base:base + nk * cap, :].rearrange(
                        "p (k c) f -> p k c f", c=cap)
                    h = cap
                    while h > 1:
                        lo = (h + 1) // 2
                        nc.vector.tensor_add(blk[:, :, 0:h - lo, :],
                                             blk[:, :, 0:h - lo, :],
                                             blk[:, :, lo:h, :])
                        h = lo
                # combine L + H roots -> agg[:, k0:k1, :]
                yL = y[:, 0:nL, :].rearrange("p (k c) f -> p k c f", c=cL) \
                    if nL else None
                yH = y[:, nL:cols, :].rearrange("p (k c) f -> p k c f", c=cH) \
                    if nH else None
                if yL is not None and yH is not None:
                    nc.vector.tensor_add(agg[:, k0:k1, :], yL[:, :, 0, :],
                                         yH[:, :, 0, :])
                elif yL is not None:
                    nc.vector.tensor_copy(agg[:, k0:k1, :], yL[:, :, 0, :])
                else:
                    nc.vector.tensor_copy(agg[:, k0:k1, :], yH[:, :, 0, :])

            ng = len(groups)
            for rep in range(reps):
                agg = rpool.tile([P, T_CORE, D + 2], F16, tag="agg")
                cur_agg[0] = agg
                gw = {0: emit_gather(0)}
                if ng > 1:
                    gw[1] = emit_gather(1)
                handles = {}
                for s in range(ng):
                    if s + 2 < ng:
                        gw[s + 2] = emit_gather(s + 2)
                    us = emit_part1(s, slvl, gw[s])
                    if s >= 1:
                        emit_part3(s - 1, slvl, handles.pop(s - 1))
                    handles[s] = emit_part2(s, slvl, gw.pop(s), us)
                emit_part3(ng - 1, slvl, handles.pop(ng - 1))
                if slvl < 4:
                    continue

                # tail: rec = 1 / max(den, eps); out = sigmoid(num * rec)
                rec = rpool.tile([P, T_CORE], F32, tag="rec")
                rec2 = rpool.tile([P, T_CORE, EXW], F16, tag="rec2")
                obuf = rpool.tile([P, T_CORE, D], F16, tag="obuf")
                nc.vector.tensor_scalar_max(rec[:], agg[:, :, D], 1e-9)
                nc.vector.reciprocal(rec[:], rec[:])
                nc.scalar.activation(
                    rec2[:],
                    rec[:].unsqueeze(2).to_broadcast((P, T_CORE, EXW)),
                    AF.Identity)
                nc.vector.tensor_mul(
                    obuf[:].rearrange("p t (a b) -> p t a b", b=EXW),
                    agg[:, :, 0:D].rearrange("p t (a b) -> p t a b", b=EXW),
                    rec2[:, :, 0:EXW].unsqueeze(2)
                    .to_broadcast((P, T_CORE, D // EXW, EXW)))
                nc.vector.tensor_mul(
                    obuf[:], obuf[:],
                    inva_sb[:].unsqueeze(1).to_broadcast((P, T_CORE, D)))
                nc.scalar.activation(obuf[:], obuf[:], AF.Sigmoid)
                nc.sync.dma_start(
                    out_d[:, :], obuf[:].rearrange("p t f -> p (t f)"))

    nc.compile()
    return nc


_CACHE = {}


def kernel(x, W, b, a, edge_index):
    x = np.ascontiguousarray(np.asarray(x, dtype=np.float32))
    W = np.ascontiguousarray(np.asarray(W, dtype=np.float32))
    b = np.ascontiguousarray(np.asarray(b, dtype=np.float32))
    a = np.ascontiguousarray(np.asarray(a, dtype=np.float32))
    edge_index = np.asarray(edge_index)

    cfg, in_maps, meta = prepare(x, W, b, a, edge_index)
    nc = _CACHE.get(cfg)
    if nc is None:
        nc = build(cfg)
        _CACHE[cfg] = nc

    from concourse.bass_utils import run_bass_kernel_spmd
    res = run_bass_kernel_spmd(nc, in_maps, core_ids=list(range(N_CORES)))

    N = meta["N"]
    fperm = meta["fperm"]
    tiles = meta["tiles"]
    tile_of = meta["tile_of"]
    inv_f = np.argsort(fperm)
    y = np.empty((NP_, D), np.float32)
    for c in range(N_CORES):
        o = np.asarray(res.results[c]["out"]).reshape(P, T_CORE, D)
        own = tiles[tile_of[:, c]]              # [50, 128]; o[p,k]=own[k,p]
        y[own.transpose(1, 0).reshape(-1)] = o.reshape(-1, D)
    return y[:N][:, inv_f].astype(np.float32)
